# revision 6
# baseline (speedup 1.0000x reference)
"""Multi-latent attention (B=2,T=2048,C=1024,H=16,HD=64,L=8) on 8 NeuronCores.

Sharding: core c -> (b = c//4, head-group g = c%4 of 4 consecutive heads).
Each core computes q/k/v projections for its 4 heads (tensor-parallel columns),
RoPE, causal attention with 8 latent "sink" keys (latent values are zero, so
latents only contribute to the softmax denominator), and a partial output
projection y_partial = attn_out @ Wproj[rows of its heads].  The host sums the
4 partial projections per batch element.

Device scheme per core:
  - xT (C on partitions) is produced host-side; q/k are projected directly into
    head-transposed layout (head-dim on partitions) with RoPE even/odd
    de-interleave folded into the Wq/Wk column order; RoPE itself is 4 vector
    ops per tile using mixed PSUM/SBUF operands.
  - scores are computed transposed (k on partitions, q free) so softmax needs
    no cross-partition max (exp is safe unshifted here); exp'd scores then act
    as matmul weights against v(+ones column) giving attention output with q
    back on partitions and the softmax denominator in the ones column;
    normalization is a per-partition tensor_scalar; a 128x64 PE transpose puts
    the result feature-major for the output projection.
"""

import math
import numpy as np
import ml_dtypes

import concourse.bass as bass
import concourse.mybir as mybir
from concourse import bacc
from concourse.tile import TileContext
from concourse.alu_op_type import AluOpType
from concourse.bass_utils import run_bass_kernel_spmd

F32 = mybir.dt.float32
BF16 = mybir.dt.bfloat16
EXP = mybir.ActivationFunctionType.Exp

B, T, C = 2, 2048, 1024
H, HD, L, LD = 16, 64, 8, 128
THETA = 10000.0
HPC = 4            # heads per core
NT = T // 128      # 16 token tiles
NCC = C // 128     # 8 contraction chunks
QC = T // 512      # 4 query chunks of 512
SCALE = 1.0 / math.sqrt(HD)
NEG = -1.0e9

_cache = {}


def _build_program():
    nc = bacc.Bacc("TRN2", target_bir_lowering=False, debug=False, num_devices=8)

    xT = nc.dram_tensor("xT", [C, T], BF16, kind="ExternalInput").ap()
    wq = nc.dram_tensor("wq", [C, 256], BF16, kind="ExternalInput").ap()
    wk = nc.dram_tensor("wk", [C, 256], BF16, kind="ExternalInput").ap()
    wv = nc.dram_tensor("wv", [C, 256], BF16, kind="ExternalInput").ap()
    wp = nc.dram_tensor("wp", [256, C], BF16, kind="ExternalInput").ap()
    cosF = nc.dram_tensor("cosF", [128, T], F32, kind="ExternalInput").ap()
    sinF = nc.dram_tensor("sinF", [128, T], F32, kind="ExternalInput").ap()
    lkT = nc.dram_tensor("lkT", [64, HPC * L], BF16, kind="ExternalInput").ap()
    mask = nc.dram_tensor("mask", [128, 128], F32, kind="ExternalInput").ap()
    ident = nc.dram_tensor("ident", [128, 128], BF16, kind="ExternalInput").ap()
    y = nc.dram_tensor("y", [T, C], F32, kind="ExternalOutput").ap()

    with TileContext(nc) as tc:
        with tc.tile_pool(name="const", bufs=1) as cpool, \
             tc.tile_pool(name="xtp", bufs=1) as xtp, \
             tc.tile_pool(name="wqkv", bufs=1) as wpool, \
             tc.tile_pool(name="qk_sb", bufs=1) as qkpool, \
             tc.tile_pool(name="v_sb", bufs=1) as vpool, \
             tc.tile_pool(name="atto", bufs=1) as apool:

            # ---- constant / weight loads ----
            cos_t = cpool.tile([128, T], F32, tag="cos")
            sin_t = cpool.tile([128, T], F32, tag="sin")
            nc.sync.dma_start(out=cos_t[:, :], in_=cosF[:, :])
            nc.sync.dma_start(out=sin_t[:, :], in_=sinF[:, :])
            mask_t = cpool.tile([128, 128], F32, tag="mask")
            nc.sync.dma_start(out=mask_t[:, :], in_=mask[:, :])
            id_t = cpool.tile([128, 128], BF16, tag="ident")
            nc.sync.dma_start(out=id_t[:, :], in_=ident[:, :])
            lk_t = cpool.tile([128, HPC * L], BF16, tag="lk")
            nc.sync.dma_start(out=lk_t[0:64, :], in_=lkT[:, :])
            nc.sync.dma_start(out=lk_t[64:128, :], in_=lkT[:, :])

            xt = []
            for cc in range(NCC):
                t = xtp.tile([128, T], BF16, tag=f"x{cc}")
                nc.sync.dma_start(out=t[:, :], in_=xT[cc * 128:(cc + 1) * 128, :])
                xt.append(t)
            wq_t, wk_t, wv_t = [], [], []
            for name, ext, lst in (("wq", wq, wq_t), ("wk", wk, wk_t), ("wv", wv, wv_t)):
                for cc in range(NCC):
                    t = wpool.tile([128, 256], BF16, tag=f"{name}{cc}")
                    nc.sync.dma_start(out=t[:, :], in_=ext[cc * 128:(cc + 1) * 128, :])
                    lst.append(t)
            wp_t = []
            for p in range(2):
                t = wpool.tile([128, C], BF16, tag=f"wp{p}")
                nc.sync.dma_start(out=t[:, :], in_=wp[p * 128:(p + 1) * 128, :])
                wp_t.append(t)

            qT = [qkpool.tile([128, T], BF16, tag=f"qT{p}", name=f"qT{p}") for p in range(2)]
            kT = [qkpool.tile([128, T], BF16, tag=f"kT{p}", name=f"kT{p}") for p in range(2)]
            v_sb = [vpool.tile([128, 4 * 65], BF16, tag=f"v{mt}", name=f"v{mt}") for mt in range(NT)]
            attoT = [apool.tile([128, T], BF16, tag=f"at{p}", name=f"at{p}") for p in range(2)]

            # ---- phase 1: q/k/v projections (+ RoPE on q,k) ----
            with tc.tile_pool(name="ps1", bufs=2, space="PSUM") as ps1, \
                 tc.tile_pool(name="vps", bufs=2, space="PSUM") as vps, \
                 tc.tile_pool(name="rope_ps", bufs=2, space="PSUM") as rps, \
                 tc.tile_pool(name="rope_sb", bufs=2) as rsb:
                for p in range(2):
                    for wlist, dst in ((wq_t, qT[p]), (wk_t, kT[p])):
                        for qc in range(QC):
                            ps = ps1.tile([128, 512], F32, tag="proj")
                            for cc in range(NCC):
                                nc.tensor.matmul(
                                    ps[:, :],
                                    wlist[cc][:, p * 128:(p + 1) * 128],
                                    xt[cc][:, qc * 512:(qc + 1) * 512],
                                    start=(cc == 0), stop=(cc == NCC - 1))
                            # RoPE: m1 = ps*cos (SBUF), m2 = ps*sin (PSUM)
                            # out_e = m1[e] - m2[o];  out_o = m1[o] + m2[e]
                            cs = cos_t[:, qc * 512:(qc + 1) * 512]
                            sn = sin_t[:, qc * 512:(qc + 1) * 512]
                            m1 = rsb.tile([128, 512], F32, tag="m1")
                            m2 = rps.tile([128, 512], F32, tag="m2")
                            nc.vector.tensor_tensor(m1[:, :], ps[:, :], cs, AluOpType.mult)
                            nc.vector.tensor_tensor(m2[:, :], ps[:, :], sn, AluOpType.mult)
                            o = dst[:, qc * 512:(qc + 1) * 512]
                            for hb in (0, 64):
                                nc.vector.tensor_tensor(
                                    o[hb:hb + 32, :], m1[hb:hb + 32, :],
                                    m2[hb + 32:hb + 64, :], AluOpType.subtract)
                                nc.vector.tensor_tensor(
                                    o[hb + 32:hb + 64, :], m1[hb + 32:hb + 64, :],
                                    m2[hb:hb + 32, :], AluOpType.add)

                # v: token-major (stationary = xT chunk, moving = wv)
                for mt in range(NT):
                    ps = vps.tile([128, 256], F32, tag="vproj")
                    for cc in range(NCC):
                        nc.tensor.matmul(
                            ps[:, :],
                            xt[cc][:, mt * 128:(mt + 1) * 128],
                            wv_t[cc][:, :],
                            start=(cc == 0), stop=(cc == NCC - 1))
                    for h in range(HPC):
                        nc.vector.tensor_copy(
                            v_sb[mt][:, h * 65:h * 65 + 64],
                            ps[:, h * 64:(h + 1) * 64])
                    nc.vector.memset(v_sb[mt][:, 64:4 * 65:65], 1.0)

            # ---- phase 2: attention ----
            with tc.tile_pool(name="s_ps", bufs=2, space="PSUM") as sps, \
                 tc.tile_pool(name="lat_ps", bufs=1, space="PSUM") as lps, \
                 tc.tile_pool(name="av_ps", bufs=2, space="PSUM") as avps, \
                 tc.tile_pool(name="tr_ps", bufs=2, space="PSUM") as tps, \
                 tc.tile_pool(name="exp_sb", bufs=1) as esb, \
                 tc.tile_pool(name="msk_sb", bufs=2) as msb, \
                 tc.tile_pool(name="d_sb", bufs=4) as dsb, \
                 tc.tile_pool(name="ao_sb", bufs=2) as aosb:
                for h in range(HPC):
                    p, hoff = h // 2, (h % 2) * 64
                    for qc in range(QC):
                        nkt = 4 * qc + 4
                        exp_tiles = []
                        for kt in range(nkt):
                            r = kt - 4 * qc
                            cs0 = 128 * r if r >= 0 else 0
                            n = 512 - cs0
                            sp = sps.tile([128, 512], F32, tag="s")
                            nc.tensor.matmul(
                                sp[:, :n],
                                kT[p][hoff:hoff + 64, kt * 128:(kt + 1) * 128],
                                qT[p][hoff:hoff + 64,
                                      qc * 512 + cs0:(qc + 1) * 512],
                                start=True, stop=True)
                            ex = esb.tile([128, 512], BF16, tag=f"exp{kt}")
                            if r >= 0:
                                mk = msb.tile([128, 128], F32, tag="mk")
                                nc.vector.scalar_tensor_tensor(
                                    mk[:, :], sp[:, 0:128], 1.0, mask_t[:, :],
                                    AluOpType.mult, AluOpType.add)
                                nc.scalar.activation(ex[:, 0:128], mk[:, :], EXP,
                                                     bias=0.0, scale=SCALE)
                                if n > 128:
                                    nc.scalar.activation(ex[:, 128:n], sp[:, 128:n],
                                                         EXP, bias=0.0, scale=SCALE)
                            else:
                                nc.scalar.activation(ex[:, :n], sp[:, :n], EXP,
                                                     bias=0.0, scale=SCALE)
                            exp_tiles.append((ex, cs0))
                        for qt4 in range(4):
                            qt = 4 * qc + qt4
                            lat = lps.tile([128, L], F32, tag="lat")
                            nc.tensor.matmul(
                                lat[:, :],
                                qT[p][hoff:hoff + 64, qt * 128:(qt + 1) * 128],
                                lk_t[hoff:hoff + 64, h * L:(h + 1) * L],
                                start=True, stop=True)
                            el = dsb.tile([128, L], BF16, tag="el")
                            lacc = dsb.tile([128, 1], F32, tag="lacc")
                            nc.scalar.activation(el[:, :], lat[:, :], EXP,
                                                 bias=0.0, scale=SCALE,
                                                 accum_out=lacc[:, :])
                            av = avps.tile([128, 65], F32, tag="av")
                            for kt in range(qt + 1):
                                ex, cs0 = exp_tiles[kt]
                                col = 128 * qt4 - cs0
                                nc.tensor.matmul(
                                    av[:, :],
                                    ex[:, col:col + 128],
                                    v_sb[kt][:, h * 65:(h + 1) * 65],
                                    start=(kt == 0), stop=(kt == qt))
                            den = dsb.tile([128, 1], F32, tag="den")
                            nc.vector.tensor_tensor(den[:, :], av[:, 64:65],
                                                    lacc[:, :], AluOpType.add)
                            invd = dsb.tile([128, 1], F32, tag="invd")
                            nc.vector.reciprocal(invd[:, :], den[:, :])
                            ao = aosb.tile([128, 64], BF16, tag="ao")
                            nc.vector.tensor_scalar_mul(ao[:, :], av[:, 0:64],
                                                        invd[:, :])
                            tp = tps.tile([64, 128], BF16, tag="tp")
                            nc.tensor.transpose(tp[:, :], ao[:, :], id_t[:, :])
                            nc.vector.tensor_copy(
                                attoT[p][hoff:hoff + 64,
                                         qt * 128:(qt + 1) * 128],
                                tp[:, :])

            # ---- phase 3: output projection (partial: this core's heads) ----
            with tc.tile_pool(name="y_ps", bufs=2, space="PSUM") as yps, \
                 tc.tile_pool(name="y_sb", bufs=3) as ysb:
                for mt in range(NT):
                    for nn in range(2):
                        yp = yps.tile([128, 512], F32, tag="y")
                        for p in range(2):
                            nc.tensor.matmul(
                                yp[:, :],
                                attoT[p][:, mt * 128:(mt + 1) * 128],
                                wp_t[p][:, nn * 512:(nn + 1) * 512],
                                start=(p == 0), stop=(p == 1))
                        ys = ysb.tile([128, 512], F32, tag="ys")
                        nc.vector.tensor_copy(ys[:, :], yp[:, :])
                        nc.sync.dma_start(
                            out=y[mt * 128:(mt + 1) * 128, nn * 512:(nn + 1) * 512],
                            in_=ys[:, :])

    nc.compile()
    return nc


def _deinterleave_cols(w):
    # (C, 64) per head -> [even d cols | odd d cols]
    return np.concatenate([w[:, 0::2], w[:, 1::2]], axis=1)


def _host_prep(x, Wq, Wk, Wv, lat_k, Wlk, Wproj):
    bf = ml_dtypes.bfloat16
    freqs = 1.0 / (THETA ** (np.arange(0, HD, 2, dtype=np.float64) / HD))
    ang = np.arange(T, dtype=np.float64)[:, None] * freqs[None, :]
    cos32 = np.cos(ang).T.astype(np.float32)     # (32, T)
    sin32 = np.sin(ang).T.astype(np.float32)
    cosF = np.concatenate([cos32] * 4, axis=0)
    sinF = np.concatenate([sin32] * 4, axis=0)

    mask = np.triu(np.full((128, 128), NEG, np.float32), 1).T.copy()
    identity = np.eye(128, dtype=bf)

    lk = (lat_k[0].astype(np.float64) @ Wlk.astype(np.float64)).astype(np.float32)
    lk = lk.reshape(L, H, HD)                     # (8, 16, 64)

    maps = []
    for core in range(8):
        b, g = core // 4, core % 4
        hs = [4 * g + i for i in range(HPC)]
        wq_c = np.concatenate(
            [_deinterleave_cols(Wq[:, h * HD:(h + 1) * HD]) for h in hs], axis=1)
        wk_c = np.concatenate(
            [_deinterleave_cols(Wk[:, h * HD:(h + 1) * HD]) for h in hs], axis=1)
        wv_c = np.concatenate([Wv[:, h * HD:(h + 1) * HD] for h in hs], axis=1)
        wp_c = Wproj[g * 256:(g + 1) * 256, :]
        lkT_c = np.concatenate(
            [np.concatenate([lk[:, h, 0::2], lk[:, h, 1::2]], axis=1).T for h in hs],
            axis=1)                               # (64, 32)
        maps.append({
            "xT": np.ascontiguousarray(x[b].T).astype(bf),
            "wq": wq_c.astype(bf),
            "wk": wk_c.astype(bf),
            "wv": wv_c.astype(bf),
            "wp": wp_c.astype(bf),
            "cosF": cosF,
            "sinF": sinF,
            "lkT": lkT_c.astype(bf),
            "mask": mask,
            "ident": identity,
        })
    return maps


def kernel(x, Wq, Wk, Wv, lat_q, lat_k, Wlq, Wlk, Wproj):
    if "nc" not in _cache:
        _cache["nc"] = _build_program()
    nc = _cache["nc"]
    maps = _host_prep(np.asarray(x, np.float32), np.asarray(Wq, np.float32),
                      np.asarray(Wk, np.float32), np.asarray(Wv, np.float32),
                      np.asarray(lat_k, np.float32), np.asarray(Wlk, np.float32),
                      np.asarray(Wproj, np.float32))
    res = run_bass_kernel_spmd(nc, maps, list(range(8)))
    out = np.zeros((B, T, C), np.float32)
    for core in range(8):
        out[core // 4] += res.results[core]["y"]
    return out
